# revision 1
# baseline (speedup 1.0000x reference)
"""Trainium2 Bass kernel for a PointNet-style neighborhood encoder.

Computation (matches the reference nn.Module):
    h = relu(relu(relu(points @ W0 + b0) @ W1 + b1) @ W2 + b2)   # [N,3] -> [N,128]
    pooled = segment_max(h, cluster)                             # [C,128], 32 pts/cluster
    out = relu(relu(pooled @ G0 + g0) @ G1 + g1)                 # [C,256]

Sharding: data-parallel over points across 8 NeuronCores (cluster
boundaries are shard-aligned because clusters are contiguous, 32
points each). Weights are replicated. No collectives; the host
scatters inputs and gathers per-core outputs.

Device strategy (per core, n = 262144 points = 65536 quad-columns):
  - Host packs points feature-major, 4 points per 128-partition column
    ("quads"): pts4[3a+f, q] = points[4q+a, f], so layer 0 is a single
    block-diagonal matmul (K=12, M=128) producing h0 for 4 points/col.
  - Layer 1 uses two permuted block-diagonal stationaries W1A/W1B
    (K=128, M=128) producing h1 with 2 points per column.
  - Layer 2 uses W2 duplicated on both partition halves; 4 sub-matmuls
    (K=64, M=128) with rhs partition slices map to distinct PE row
    groups, producing z = W2^T h1 (bias/relu deferred) in PSUM.
  - segment_max: relu is monotone and b2 is constant per feature, so
    pooled = relu(max_p(z) + b2). max over (4 tensors x 8 quads) is ONE
    VectorE tensor_reduce(axis=XY) straight out of PSUM per sub-chunk.
  - ScalarE (ACT) does every relu+bias PSUM->SBUF evacuation; VectorE
    only does the pooling reduces. bf16 activations everywhere
    (PSUM stays f32 as the HW requires).
  - Global MLP on pooled [128, 8192] per core; output is written
    feature-major [256, 8192] bf16 and transposed/upcast on the host.
"""

import numpy as np

# ---- problem geometry (hardcoded per contract) ----
N = 2097152          # total points
C = 65536            # clusters
PTS = 32             # points per cluster
NCORES = 8
NPC = N // NCORES    # points per core = 262144
N4C = NPC // 4       # quad-columns per core = 65536
CPC = C // NCORES    # clusters per core = 8192

BIG = 1024           # quad-columns per big-chunk
SUB = 256            # quad-columns per L2/pool sub-chunk
NCHUNK = N4C // BIG  # 64
NSUB = BIG // SUB    # 4

_CACHE = {}


def _bf16():
    import ml_dtypes
    return ml_dtypes.bfloat16


def _build_module(n4c: int):
    """Build the Bass module (SPMD program, same for all cores)."""
    import concourse.bass as bass
    import concourse.bacc as bacc
    import concourse.tile as tile
    from concourse import mybir

    BF = mybir.dt.bfloat16
    F32 = mybir.dt.float32
    RELU = mybir.ActivationFunctionType.Relu
    MAX = mybir.AluOpType.max
    XY = mybir.AxisListType.XY

    nchunk = n4c // BIG
    cpc = n4c // 8          # clusters per core for this size

    nc = bacc.Bacc()

    # ---- DRAM I/O ----
    pts4 = nc.dram_tensor("pts4", [12, n4c], BF, kind="ExternalInput")
    w0q = nc.dram_tensor("w0q", [12, 128], BF, kind="ExternalInput")
    w1a = nc.dram_tensor("w1a", [128, 128], BF, kind="ExternalInput")
    w1b = nc.dram_tensor("w1b", [128, 128], BF, kind="ExternalInput")
    w2d = nc.dram_tensor("w2d", [128, 128], BF, kind="ExternalInput")
    g0w = nc.dram_tensor("g0w", [128, 128], BF, kind="ExternalInput")
    g1lo = nc.dram_tensor("g1lo", [128, 128], BF, kind="ExternalInput")
    g1hi = nc.dram_tensor("g1hi", [128, 128], BF, kind="ExternalInput")
    b0q = nc.dram_tensor("b0q", [128, 1], F32, kind="ExternalInput")
    b1d = nc.dram_tensor("b1d", [128, 1], F32, kind="ExternalInput")
    b2v = nc.dram_tensor("b2v", [128, 1], F32, kind="ExternalInput")
    g0v = nc.dram_tensor("g0v", [128, 1], F32, kind="ExternalInput")
    g1l = nc.dram_tensor("g1l", [128, 1], F32, kind="ExternalInput")
    g1h = nc.dram_tensor("g1h", [128, 1], F32, kind="ExternalInput")
    outt = nc.dram_tensor("outt", [256, cpc], BF, kind="ExternalOutput")

    from contextlib import ExitStack
    with tile.TileContext(nc) as tc, ExitStack() as ctx:
        singles = ctx.enter_context(tc.tile_pool(name="singles", bufs=1))
        ppts = ctx.enter_context(tc.tile_pool(name="ppts", bufs=3))
        ph0s = ctx.enter_context(tc.tile_pool(name="ph0s", bufs=2))
        ph1s = ctx.enter_context(tc.tile_pool(name="ph1s", bufs=2))
        psum_h = ctx.enter_context(tc.tile_pool(name="psum_h", bufs=2, space="PSUM"))
        psum_h1 = ctx.enter_context(tc.tile_pool(name="psum_h1", bufs=1, space="PSUM"))
        psum_z = ctx.enter_context(tc.tile_pool(name="psum_z", bufs=2, space="PSUM"))

        # ---- load constants ----
        w0q_s = singles.tile([12, 128], BF)
        w1a_s = singles.tile([128, 128], BF)
        w1b_s = singles.tile([128, 128], BF)
        w2d_s = singles.tile([128, 128], BF)
        g0w_s = singles.tile([128, 128], BF)
        g1lo_s = singles.tile([128, 128], BF)
        g1hi_s = singles.tile([128, 128], BF)
        for dst, src in ((w0q_s, w0q), (w1a_s, w1a), (w1b_s, w1b),
                         (w2d_s, w2d), (g0w_s, g0w), (g1lo_s, g1lo),
                         (g1hi_s, g1hi)):
            nc.gpsimd.dma_start(out=dst[:], in_=src[:])
        b0q_s = singles.tile([128, 1], F32)
        b1d_s = singles.tile([128, 1], F32)
        b2v_s = singles.tile([128, 1], F32)
        g0v_s = singles.tile([128, 1], F32)
        g1l_s = singles.tile([128, 1], F32)
        g1h_s = singles.tile([128, 1], F32)
        for dst, src in ((b0q_s, b0q), (b1d_s, b1d), (b2v_s, b2v),
                         (g0v_s, g0v), (g1l_s, g1l), (g1h_s, g1h)):
            nc.gpsimd.dma_start(out=dst[:], in_=src[:])

        # pooled max(z) accumulator for the whole core
        pooled = singles.tile([128, cpc], BF)

        # ---- main loop over point chunks (L2/pool shifted one chunk) ----
        def emit_l2_pair(i, jpair, h1as, h1bs):
            """L2 matmuls for subs (2*jpair, 2*jpair+1) of chunk i, weight-
            batched (both lo-half MMs, then both hi-half), then the pools."""
            zps = []
            for j in (2 * jpair, 2 * jpair + 1):
                s0 = j * SUB
                zp = psum_z.tile([128, 4 * SUB], F32, tag="zp")
                zps.append((j, s0, zp))
            for j, s0, zp in zps:
                nc.tensor.matmul(zp[:, 0:SUB],
                                 w2d_s[0:64, :], h1as[0:64, s0:s0 + SUB])
                nc.tensor.matmul(zp[:, SUB:2 * SUB],
                                 w2d_s[0:64, :], h1bs[0:64, s0:s0 + SUB])
            for j, s0, zp in zps:
                nc.tensor.matmul(zp[:, 2 * SUB:3 * SUB],
                                 w2d_s[64:128, :], h1as[64:128, s0:s0 + SUB])
                nc.tensor.matmul(zp[:, 3 * SUB:4 * SUB],
                                 w2d_s[64:128, :], h1bs[64:128, s0:s0 + SUB])
            for j, s0, zp in zps:
                # pooled_raw = max over (4 tensors x 8 quads) per cluster
                zv = zp.rearrange("p (t c q) -> p c t q", t=4, q=8)
                base = i * (BIG // 8) + j * (SUB // 8)
                nc.vector.tensor_reduce(
                    pooled[:, base:base + SUB // 8], zv, axis=XY, op=MAX)

        g0in = singles.tile([128, cpc], BF)
        g1in = singles.tile([128, cpc], BF)
        goutL = singles.tile([128, cpc], BF)
        goutH = singles.tile([128, cpc], BF)

        def g_task_g0(k, h):
            sl = slice(k * 512 + h * 256, k * 512 + (h + 1) * 256)
            nc.scalar.activation(g0in[:, sl], pooled[:, sl], RELU,
                                 bias=b2v_s[:])
            gp = psum_h.tile([128, 256], F32, tag="h0p")
            nc.tensor.matmul(gp[:], g0w_s[:], g0in[:, sl])
            nc.scalar.activation(g1in[:, sl], gp[:], RELU, bias=g0v_s[:])

        def g_task_lo(k, h):
            sl = slice(k * 512 + h * 256, k * 512 + (h + 1) * 256)
            gpl = psum_h.tile([128, 256], F32, tag="h0p")
            nc.tensor.matmul(gpl[:], g1lo_s[:], g1in[:, sl])
            nc.scalar.activation(goutL[:, sl], gpl[:], RELU, bias=g1l_s[:])
            nc.sync.dma_start(out=outt[0:128, sl], in_=goutL[:, sl])

        def g_task_hi(k, h):
            sl = slice(k * 512 + h * 256, k * 512 + (h + 1) * 256)
            gph = psum_h.tile([128, 256], F32, tag="h0p")
            nc.tensor.matmul(gph[:], g1hi_s[:], g1in[:, sl])
            nc.scalar.activation(goutH[:, sl], gph[:], RELU, bias=g1h_s[:])
            nc.sync.dma_start(out=outt[128:256, sl], in_=goutH[:, sl])

        def emit_g_block(k):
            for h in (0, 1):
                g_task_g0(k, h); g_task_lo(k, h); g_task_hi(k, h)

        g_tasks = []

        def pop_g_task():
            if g_tasks:
                fn, k, h = g_tasks.pop(0)
                fn(k, h)

        prev = None   # (i, h1as, h1bs) pending L2+pool
        for i in range(nchunk):
            c0 = i * BIG
            pts_t = ppts.tile([12, BIG], BF)
            nc.sync.dma_start(out=pts_t[:], in_=pts4[:, c0:c0 + BIG])

            # L0: 4-point block-diagonal matmul, K=12 -> M=128
            h0pa = psum_h.tile([128, 512], F32, tag="h0p")
            h0pb = psum_h.tile([128, 512], F32, tag="h0p")
            nc.tensor.matmul(h0pa[:], w0q_s[:], pts_t[:, 0:512])
            nc.tensor.matmul(h0pb[:], w0q_s[:], pts_t[:, 512:1024])
            h0s = ph0s.tile([128, BIG], BF)
            nc.scalar.activation(h0s[:, 0:512], h0pa[:], RELU, bias=b0q_s[:])
            nc.scalar.activation(h0s[:, 512:1024], h0pb[:], RELU, bias=b0q_s[:])

            if prev is not None:
                emit_l2_pair(prev[0], 0, prev[1], prev[2])
                pop_g_task()
                emit_l2_pair(prev[0], 1, prev[1], prev[2])

            # L1: two block-diagonal stationaries -> h1 (2 pts/col)
            h1p = psum_h1.tile([128, BIG], F32, tag="h1p")
            nc.tensor.matmul(h1p[:, 0:512], w1a_s[:], h0s[:, 0:512])
            nc.tensor.matmul(h1p[:, 512:1024], w1a_s[:], h0s[:, 512:1024])
            h1as = ph1s.tile([128, BIG], BF, tag="h1as")
            if i == 0:
                nc.scalar.activation(h1as[:, 0:512], h1p[:, 0:512], RELU,
                                     bias=b1d_s[:])
                nc.scalar.activation(h1as[:, 512:1024], h1p[:, 512:1024],
                                     RELU, bias=b1d_s[:])
            else:
                nc.scalar.activation(h1as[:], h1p[:], RELU, bias=b1d_s[:])

            if i == 0:
                # prologue: borrow an idle z slot so L1B needn't wait for
                # evac1A's PSUM release; the first L2 pair waits on h1bs
                # anyway, which releases the slot before L2 needs it.
                h1p2 = psum_z.tile([128, BIG], F32, tag="zp")
            else:
                h1p2 = psum_h1.tile([128, BIG], F32, tag="h1p")
            nc.tensor.matmul(h1p2[:, 0:512], w1b_s[:], h0s[:, 0:512])
            nc.tensor.matmul(h1p2[:, 512:1024], w1b_s[:], h0s[:, 512:1024])
            h1bs = ph1s.tile([128, BIG], BF, tag="h1bs")
            if i == 0:
                # split so the first L2 pair (cols 0:512) unblocks sooner
                nc.scalar.activation(h1bs[:, 0:512], h1p2[:, 0:512], RELU,
                                     bias=b1d_s[:])
                nc.scalar.activation(h1bs[:, 512:1024], h1p2[:, 512:1024],
                                     RELU, bias=b1d_s[:])
            else:
                nc.scalar.activation(h1bs[:], h1p2[:], RELU, bias=b1d_s[:])

            prev = (i, h1as, h1bs)

            # interleave global-MLP work once pooled slices complete:
            # block k (clusters 512k..512k+512) is pooled after iteration
            # 4k+4 starts (the shifted L2 of chunk 4k+3 was emitted above).
            if i >= 4 and (i - 4) % 4 == 0:
                k = (i - 4) // 4
                g_tasks.extend([(f, k, h) for h in (0, 1)
                                for f in (g_task_g0, g_task_lo, g_task_hi)])
            pop_g_task()

        # epilogue: overlap the final global-MLP work with the last pools.
        # Half-block (k, 0) only needs chunks 4k..4k+1 (pooled in-loop), so
        # everything except the very last half-block can run alongside the
        # final L2 pairs; (last_k, 1) needs the last chunk's pools.
        emit_l2_pair(prev[0], 0, prev[1], prev[2])
        for fn, k, h in g_tasks:
            fn(k, h)
        first_unpushed = ((nchunk - 5) // 4 + 1) if nchunk >= 5 else 0
        last_k = cpc // 512 - 1
        for k in range(first_unpushed, last_k + 1):
            for f in (g_task_g0, g_task_lo, g_task_hi):
                f(k, 0)
            if k < last_k:
                for f in (g_task_g0, g_task_lo, g_task_hi):
                    f(k, 1)
        emit_l2_pair(prev[0], 1, prev[1], prev[2])
        for f in (g_task_g0, g_task_lo, g_task_hi):
            f(last_k, 1)

    nc.compile()
    return nc


def _host_pack(points, W0, b0, W1, b1, W2, b2, G0, g0, G1, g1, n4c):
    """Build per-core input maps (host-side layout prep, numpy only)."""
    bf16 = _bf16()
    n = n4c * 4 * NCORES

    # pts4[3a+f, q] = points[4q+a, f]
    pts4 = np.ascontiguousarray(
        points[:n].reshape(-1, 4, 3).transpose(1, 2, 0).reshape(12, -1)
    ).astype(bf16)

    # W0 block-diagonal over 4 points: [12, 128]
    w0q = np.zeros((12, 128), np.float32)
    for a in range(4):
        w0q[3 * a:3 * a + 3, 32 * a:32 * a + 32] = W0
    # W1A/W1B: rows 32a+f; cols 64a'+g ; a' in {0,1} / {2,3}
    w1a = np.zeros((128, 128), np.float32)
    w1b = np.zeros((128, 128), np.float32)
    for a in range(2):
        w1a[32 * a:32 * a + 32, 64 * a:64 * a + 64] = W1
        w1b[32 * (a + 2):32 * (a + 2) + 32, 64 * a:64 * a + 64] = W1
    # W2 duplicated on both partition halves
    w2d = np.concatenate([W2, W2], axis=0)

    common = {
        "w0q": w0q.astype(bf16),
        "w1a": w1a.astype(bf16),
        "w1b": w1b.astype(bf16),
        "w2d": w2d.astype(bf16),
        "g0w": G0.astype(bf16),
        "g1lo": G1[:, :128].astype(bf16),
        "g1hi": G1[:, 128:].astype(bf16),
        "b0q": np.tile(b0, 4).reshape(128, 1).astype(np.float32),
        "b1d": np.tile(b1, 2).reshape(128, 1).astype(np.float32),
        "b2v": b2.reshape(128, 1).astype(np.float32),
        "g0v": g0.reshape(128, 1).astype(np.float32),
        "g1l": g1[:128].reshape(128, 1).astype(np.float32),
        "g1h": g1[128:].reshape(128, 1).astype(np.float32),
    }
    in_maps = []
    for c in range(NCORES):
        m = dict(common)
        m["pts4"] = np.ascontiguousarray(pts4[:, c * n4c:(c + 1) * n4c])
        in_maps.append(m)
    return in_maps


def _numpy_fallback(points, cluster, num_clusters,
                    W0, b0, W1, b1, W2, b2, G0, g0, G1, g1):
    h = points.astype(np.float32)
    for W, b in ((W0, b0), (W1, b1), (W2, b2)):
        h = np.maximum(h @ W + b, 0.0)
    order = np.argsort(cluster, kind="stable")
    cs = cluster[order]
    hs = h[order]
    starts = np.searchsorted(cs, np.arange(num_clusters), side="left")
    counts = np.bincount(cs, minlength=num_clusters)
    safe_starts = np.minimum(starts, max(len(hs) - 1, 0))
    seg = np.maximum.reduceat(hs, safe_starts, axis=0)
    seg[counts == 0] = -np.inf   # match segment_max identity on empties
    pooled = seg
    gx = pooled
    for W, b in ((G0, g0), (G1, g1)):
        gx = np.maximum(gx @ W + b, 0.0)
    return gx.astype(np.float32)


def kernel(**inputs) -> np.ndarray:
    points = np.asarray(inputs["points"], np.float32)
    cluster = np.asarray(inputs["cluster"]).astype(np.int64)
    num_clusters = int(np.asarray(inputs["num_clusters"]))
    W0 = np.asarray(inputs["W0"], np.float32); b0 = np.asarray(inputs["b0"], np.float32)
    W1 = np.asarray(inputs["W1"], np.float32); b1 = np.asarray(inputs["b1"], np.float32)
    W2 = np.asarray(inputs["W2"], np.float32); b2 = np.asarray(inputs["b2"], np.float32)
    G0 = np.asarray(inputs["G0"], np.float32); g0 = np.asarray(inputs["g0"], np.float32)
    G1 = np.asarray(inputs["G1"], np.float32); g1 = np.asarray(inputs["g1"], np.float32)

    expected = (points.shape == (N, 3) and num_clusters == C
                and cluster.shape == (N,))
    if expected:
        # contiguous equal clusters of 32 points, as produced by setup_inputs
        expected = bool(
            np.array_equal(cluster[::PTS], np.arange(C, dtype=np.int64))
            and np.array_equal(cluster, np.repeat(cluster[::PTS], PTS))
        )
    if not expected:
        return _numpy_fallback(points, cluster, num_clusters,
                               W0, b0, W1, b1, W2, b2, G0, g0, G1, g1)

    from concourse.bass_utils import run_bass_kernel_spmd

    if "nc" not in _CACHE:
        _CACHE["nc"] = _build_module(N4C)
    nc = _CACHE["nc"]

    in_maps = _host_pack(points, W0, b0, W1, b1, W2, b2, G0, g0, G1, g1, N4C)
    res = run_bass_kernel_spmd(nc, in_maps, core_ids=list(range(NCORES)))
    outs = []
    for c in range(NCORES):
        o = np.asarray(res.results[c]["outt"]).astype(np.float32)  # [256, CPC]
        outs.append(o.T)                                           # [CPC, 256]
    return np.ascontiguousarray(np.concatenate(outs, axis=0))



# revision 43
# speedup vs baseline: 1.0926x; 1.0926x over previous
"""Trainium2 Bass kernel for a PointNet-style neighborhood encoder.

Computation (matches the reference nn.Module):
    h = relu(relu(relu(points @ W0 + b0) @ W1 + b1) @ W2 + b2)   # [N,3] -> [N,128]
    pooled = segment_max(h, cluster)                             # [C,128], 32 pts/cluster
    out = relu(relu(pooled @ G0 + g0) @ G1 + g1)                 # [C,256]

Sharding: data-parallel over points across 8 NeuronCores (cluster
boundaries are shard-aligned: clusters are contiguous, 32 points each).
Weights replicated; no collectives.

Device strategy (per core, n = 262144 points = 65536 quad-columns,
processed in 128 chunks of 512 quad-columns = 2048 points = 64 clusters):
  - Host packs points 4-per-column with interleaved ones-rows ("ptsb",
    16 rows: 4 x (3 feats + 1.0)), so layer 0 is one block-diagonal
    matmul with bias folded into the stationary; its relu runs on DVE.
  - Layer 1 (bf16): two permuted block-diagonal stationaries fill one
    [128,1024] PSUM tile (2 pts/col); a single ACT pass applies
    bias+relu and quantizes h1 to fp8 (e4m3).
  - Layer 2 + pool exploit max(a,b) = a + relu(b-a) so that most of the
    32-way segment max runs on the PE and ACT instead of DVE:
      zbase = W2^T p0            (fp8 DoubleRow, (W2q|W2r) k-tiles with a
                                  stride-0 broadcast rhs => full-precision
                                  weights at 0.5 cycles/col)
      d1 = z(p1)-z(p0), d2 = z(p3)-z(p2)   ([-W2;+W2] block stationaries)
      d3 = z(p2)-z(p0)                     (column-paired k-tiles)
      r12 = relu(d1|d2)  [ACT]   r3 = relu(d3 + r2 - r1)  [DVE]
      zbase += r1; d3 += r2 - r1; zbase += r3   (identity matmuls, PSUM
                                                 accumulation)
    leaving zbase = max over each 4-point group; one 8-wide DVE reduce
    completes the 32-way max. relu is monotone and b2 per-feature, so
    pooled = relu(max + b2) is applied when the global MLP reads it.
  - GPSIMD cannot access PSUM (and walrus rejects generic compute on
    it), so it only drives constant DMA loads.
  - Global MLP on pooled [128, 8192] in bf16; output written
    feature-major [256, 8192] bf16 and transposed/upcast on the host.
"""

import numpy as np

# ---- problem geometry (hardcoded per contract) ----
N = 2097152          # total points
C = 65536            # clusters
PTS = 32             # points per cluster
NCORES = 8
NPC = N // NCORES    # points per core = 262144
N4C = NPC // 4       # quad-columns per core = 65536
CPC = C // NCORES    # clusters per core = 8192

BIG = 512            # quad-columns per chunk (= 2048 points = 64 clusters)
NCHUNK = N4C // BIG  # 128

_CACHE = {}


def _bf16():
    import ml_dtypes
    return ml_dtypes.bfloat16


def _fp8():
    import ml_dtypes
    return ml_dtypes.float8_e4m3


def _build_module(n4c: int):
    """Build the Bass module (SPMD program, same for all cores)."""
    import concourse.bass as bass
    import concourse.bacc as bacc
    import concourse.tile as tile
    from concourse import mybir

    BF = mybir.dt.bfloat16
    F32 = mybir.dt.float32
    FP8 = mybir.dt.float8e4
    RELU = mybir.ActivationFunctionType.Relu
    MAX = mybir.AluOpType.max
    ADD = mybir.AluOpType.add
    X = mybir.AxisListType.X
    DR = mybir.MatmulPerfMode.DoubleRow

    nchunk = n4c // BIG
    cpc = n4c // 8             # clusters per core for this size
    nkblk = max(cpc // 512, 1)
    kwid = min(512, cpc)       # G-stage block width in clusters

    nc = bacc.Bacc()

    # ---- DRAM I/O ----
    ptsb = nc.dram_tensor("ptsb", [16, n4c], BF, kind="ExternalInput")
    w0b = nc.dram_tensor("w0b", [16, 128], BF, kind="ExternalInput")
    w1a = nc.dram_tensor("w1a", [128, 128], BF, kind="ExternalInput")
    w1b = nc.dram_tensor("w1b", [128, 128], BF, kind="ExternalInput")
    w2zb = nc.dram_tensor("w2zb", [64, 256], FP8, kind="ExternalInput")
    w2dd = nc.dram_tensor("w2dd", [128, 256], FP8, kind="ExternalInput")
    w2d3 = nc.dram_tensor("w2d3", [64, 512], FP8, kind="ExternalInput")
    eye2 = nc.dram_tensor("eye2", [128, 256], BF, kind="ExternalInput")
    g0w = nc.dram_tensor("g0w", [128, 128], BF, kind="ExternalInput")
    g1lo = nc.dram_tensor("g1lo", [128, 128], BF, kind="ExternalInput")
    g1hi = nc.dram_tensor("g1hi", [128, 128], BF, kind="ExternalInput")
    b1d = nc.dram_tensor("b1d", [128, 1], F32, kind="ExternalInput")
    b2v = nc.dram_tensor("b2v", [128, 1], F32, kind="ExternalInput")
    g0v = nc.dram_tensor("g0v", [128, 1], F32, kind="ExternalInput")
    g1l = nc.dram_tensor("g1l", [128, 1], F32, kind="ExternalInput")
    g1h = nc.dram_tensor("g1h", [128, 1], F32, kind="ExternalInput")
    outt = nc.dram_tensor("outt", [256, cpc], BF, kind="ExternalOutput")

    from contextlib import ExitStack
    with tile.TileContext(nc) as tc, ExitStack() as ctx:
        singles = ctx.enter_context(tc.tile_pool(name="singles", bufs=1))
        ppts = ctx.enter_context(tc.tile_pool(name="ppts", bufs=4))
        ph0 = ctx.enter_context(tc.tile_pool(name="ph0", bufs=3))
        ph1 = ctx.enter_context(tc.tile_pool(name="ph1", bufs=3))
        pr12 = ctx.enter_context(tc.tile_pool(name="pr12", bufs=2))
        pr3 = ctx.enter_context(tc.tile_pool(name="pr3", bufs=2))
        pg = ctx.enter_context(tc.tile_pool(name="pg", bufs=2))
        ppool = ctx.enter_context(tc.tile_pool(name="ppool", bufs=3))
        # PSUM: 8 banks of [128,512]f32:
        #   1 h0p + 2 h1p + 1 zbase + 2 d12 + 1 d3 + 1 G
        psum_h0 = ctx.enter_context(tc.tile_pool(name="psum_h0", bufs=1, space="PSUM"))
        psum_h1 = ctx.enter_context(tc.tile_pool(name="psum_h1", bufs=1, space="PSUM"))
        psum_zb = ctx.enter_context(tc.tile_pool(name="psum_zb", bufs=1, space="PSUM"))
        psum_dd = ctx.enter_context(tc.tile_pool(name="psum_dd", bufs=1, space="PSUM"))
        psum_d3 = ctx.enter_context(tc.tile_pool(name="psum_d3", bufs=1, space="PSUM"))
        psum_g = ctx.enter_context(tc.tile_pool(name="psum_g", bufs=1, space="PSUM"))

        # ---- constants ----
        w0b_s = singles.tile([16, 128], BF)
        w1a_s = singles.tile([128, 128], BF)
        w1b_s = singles.tile([128, 128], BF)
        w2zb_s = singles.tile([64, 256], FP8)
        w2dd_s = singles.tile([128, 256], FP8)
        w2d3_s = singles.tile([64, 512], FP8)
        eye2_s = singles.tile([128, 256], BF)
        g0w_s = singles.tile([128, 128], BF)
        g1lo_s = singles.tile([128, 128], BF)
        g1hi_s = singles.tile([128, 128], BF)
        b1d_s = singles.tile([128, 1], F32)
        b2v_s = singles.tile([128, 1], F32)
        g0v_s = singles.tile([128, 1], F32)
        g1l_s = singles.tile([128, 1], F32)
        g1h_s = singles.tile([128, 1], F32)
        # constants needed in the first windows, spread across DMA queues
        nc.sync.dma_start(out=w0b_s[:], in_=w0b[:])
        nc.scalar.dma_start(out=b1d_s[:], in_=b1d[:])
        nc.scalar.dma_start(out=w1a_s[:], in_=w1a[:])
        nc.scalar.dma_start(out=w1b_s[:], in_=w1b[:])
        nc.gpsimd.dma_start(out=w2zb_s[:], in_=w2zb[:])
        nc.gpsimd.dma_start(out=w2dd_s[:], in_=w2dd[:])
        nc.gpsimd.dma_start(out=w2d3_s[:], in_=w2d3[:])
        nc.gpsimd.dma_start(out=eye2_s[:], in_=eye2[:])

        def load_g_consts():
            for dst, src in ((g0w_s, g0w), (g1lo_s, g1lo), (g1hi_s, g1hi),
                             (b2v_s, b2v), (g0v_s, g0v), (g1l_s, g1l),
                             (g1h_s, g1h)):
                nc.gpsimd.dma_start(out=dst[:], in_=src[:])

        # warm the ACT function table during the constant-load phase
        warm = singles.tile([128, 1], F32)
        nc.vector.memset(warm[:], 0.0)
        nc.scalar.activation(warm[:], warm[:], RELU)

        # stationary views ([K, 2, M] k-tile pairs)
        v_zb = w2zb_s[:].rearrange("p (t m) -> p t m", t=2)    # (W2q | W2r)
        v_dd = w2dd_s[:].rearrange("p (t m) -> p t m", t=2)    # [-W2;+W2] q|r
        v_d3a = w2d3_s[:, 0:256].rearrange("p (t m) -> p t m", t=2)  # -+W2q
        v_d3b = w2d3_s[:, 256:512].rearrange("p (t m) -> p t m", t=2)  # -+W2r
        eyeP = eye2_s[:, 0:128]
        eyeN = eye2_s[:, 128:256]

        # pooled max(z), one tile per 512-cluster block (pre-bias/pre-relu)
        pooled_tiles = {}
        cpb = max(kwid // 64, 1)   # chunks per block (8 at full width)

        # ---- global MLP task machinery; tasks are (block, col0, width) ----
        def g_task_a(kb, c0, w):
            g0in = pg.tile([128, kwid], BF, tag="g0in")
            nc.vector.tensor_scalar(g0in[:, 0:w],
                                    pooled_tiles[kb][:, c0:c0 + w],
                                    b2v_s[:], 0.0, op0=ADD, op1=MAX)
            return g0in

        def g_task_b(kb, c0, w, g0in):
            gp0 = psum_g.tile([128, kwid], F32, tag="gp")
            nc.tensor.matmul(gp0[:, 0:w], g0w_s[:], g0in[:, 0:w])
            g1in = pg.tile([128, kwid], BF, tag="g1in")
            nc.scalar.activation(g1in[:, 0:w], gp0[:, 0:w], RELU,
                                 bias=g0v_s[:])
            return g1in

        g_tasks = []        # queue of (kind, block, col0, width) entries
        g_state = {}

        def g_task_c1(kb, c0, w, g1in):
            sl = slice(kb * kwid + c0, kb * kwid + c0 + w)
            gp1 = psum_g.tile([128, kwid], F32, tag="gp")
            nc.tensor.matmul(gp1[:, 0:w], g1lo_s[:], g1in[:, 0:w])
            goutL = pg.tile([128, kwid], BF, tag="goutL")
            nc.vector.tensor_scalar(goutL[:, 0:w], gp1[:, 0:w], g1l_s[:], 0.0,
                                    op0=ADD, op1=MAX)
            nc.sync.dma_start(out=outt[0:128, sl], in_=goutL[:, 0:w])

        def g_task_c2(kb, c0, w, g1in):
            sl = slice(kb * kwid + c0, kb * kwid + c0 + w)
            gp1 = psum_g.tile([128, kwid], F32, tag="gp")
            nc.tensor.matmul(gp1[:, 0:w], g1hi_s[:], g1in[:, 0:w])
            goutH = pg.tile([128, kwid], BF, tag="goutH")
            nc.vector.tensor_scalar(goutH[:, 0:w], gp1[:, 0:w], g1h_s[:], 0.0,
                                    op0=ADD, op1=MAX)
            nc.sync.dma_start(out=outt[128:256, sl], in_=goutH[:, 0:w])

        def push_g(kb, c0=0, w=None):
            w = kwid if w is None else w
            g_tasks.extend([(x, kb, c0, w) for x in ("a", "b", "c1", "c2")])

        def pop_g():
            if not g_tasks:
                return
            kind, kb, c0, w = g_tasks.pop(0)
            key = (kb, c0)
            if kind == "a":
                g_state[key] = g_task_a(kb, c0, w)
            elif kind == "b":
                g_state[key] = g_task_b(kb, c0, w, g_state[key])
            elif kind == "c1":
                g_task_c1(kb, c0, w, g_state[key])
            else:
                g_task_c2(kb, c0, w, g_state.pop(key))

        # ---- per-chunk emitters ----
        def emit_l0(j):
            """Lookahead layer 0 for chunk j: DMA, matmul, relu evac [DVE].
            Bias is folded into the stationary via ones rows."""
            pts_t = ppts.tile([16, BIG], BF)
            nc.sync.dma_start(out=pts_t[:], in_=ptsb[:, j * BIG:(j + 1) * BIG])
            h0p = psum_h0.tile([128, BIG], F32, tag="h0p")
            nc.tensor.matmul(h0p[:], w0b_s[:], pts_t[:])
            h0z = ph0.tile([128, BIG], BF)
            nc.vector.tensor_scalar_max(h0z[:], h0p[:], 0.0)
            return h0z

        def emit_pool(i, h1z):
            """Layer-2 + 32-way max for chunk i (one chunk late).

            h1z [128,1024] fp8: cols 0:512 hold (p0 lo | p1 hi) per quad
            column, cols 512:1024 hold (p2 lo | p3 hi)."""
            ha = h1z[:, 0:BIG]
            zbase = psum_zb.tile([128, BIG], F32, tag="zb")
            dd = psum_dd.tile([128, 2 * BIG], F32, tag="dd")
            d3 = psum_d3.tile([128, BIG], F32, tag="d3")
            rv_lo = h1z[0:64, 0:BIG].unsqueeze(1).broadcast_to([64, 2, BIG])
            rv_a = ha.unsqueeze(1).broadcast_to([128, 2, BIG])
            rv_b = h1z[:, BIG:2 * BIG].unsqueeze(1).broadcast_to([128, 2, BIG])
            rv_d3 = h1z[0:64, :].rearrange("p (t n) -> p t n", t=2)
            nc.tensor.matmul(zbase[:], v_zb, rv_lo, perf_mode=DR,
                             start=True, stop=False)
            nc.tensor.matmul(dd[:, 0:BIG], v_dd, rv_a, perf_mode=DR)
            nc.tensor.matmul(dd[:, BIG:2 * BIG], v_dd, rv_b, perf_mode=DR)
            nc.tensor.matmul(d3[:], v_d3a, rv_d3, perf_mode=DR,
                             start=True, stop=False)
            nc.tensor.matmul(d3[:], v_d3b, rv_d3, perf_mode=DR,
                             start=False, stop=False)
            # r12 = relu(d1|d2)  [ACT, one pass]
            r12 = pr12.tile([128, 2 * BIG], BF, tag="r12")
            nc.scalar.activation(r12[:], dd[:], RELU)
            # zbase += r1 ; d3 += r2 - r1   (identity accumulate matmuls)
            nc.tensor.matmul(zbase[:], eyeP, r12[:, 0:BIG],
                             start=False, stop=False)
            nc.tensor.matmul(d3[:], eyeP, r12[:, BIG:2 * BIG],
                             start=False, stop=False)
            nc.tensor.matmul(d3[:], eyeN, r12[:, 0:BIG],
                             start=False, stop=True)
            # r3 = relu(d3)  [DVE]
            r3 = pr3.tile([128, BIG], BF, tag="r3")
            nc.vector.tensor_scalar_max(r3[:], d3[:], 0.0)
            # zbase += r3 -> per-quad-column max of its 4 points
            nc.tensor.matmul(zbase[:], eyeP, r3[:], start=False, stop=True)
            # 8-wide reduce over quad columns -> 64 clusters  [DVE]
            kb, off = i // cpb, (i % cpb) * 64
            if off == 0:
                pooled_tiles[kb] = ppool.tile([128, kwid], BF, tag="pooled",
                                              name=f"pooled_{kb}")
            zv = zbase.rearrange("p (c q) -> p c q", q=8)
            nc.vector.tensor_reduce(pooled_tiles[kb][:, off:off + 64], zv,
                                    axis=X, op=MAX)

        # ---- main software-pipelined loop (L0 runs two chunks ahead) ----
        h0z_q = [emit_l0(0)]
        if nchunk > 1:
            h0z_q.append(emit_l0(1))
        prev = None   # (i, h1z) pending L2+pool
        for i in range(nchunk):
            h0z = h0z_q.pop(0)

            # L1: both block-diagonal stationaries into one PSUM tile,
            # one ACT pass applies bias+relu and emits fp8
            h1p = psum_h1.tile([128, 2 * BIG], F32, tag="h1p")
            nc.tensor.matmul(h1p[:, 0:BIG], w1a_s[:], h0z[:])
            nc.tensor.matmul(h1p[:, BIG:2 * BIG], w1b_s[:], h0z[:])
            h1z = ph1.tile([128, 2 * BIG], FP8, tag="h1z")
            nc.scalar.activation(h1z[:], h1p[:], RELU, bias=b1d_s[:])

            # L2 + pool for the previous chunk
            if prev is not None:
                emit_pool(prev[0], prev[1])

            # L0 lookahead for chunk i+2
            if i + 2 < nchunk:
                h0z_q.append(emit_l0(i + 2))
            if i == min(2, nchunk - 1):
                load_g_consts()

            prev = (i, h1z)

            # global MLP: block kb (512 clusters) needs chunks 8kb..8kb+7;
            # chunk 8kb+7's pool is emitted at iteration 8kb+8. 4 tasks per
            # block, popped every other chunk. The LAST block is split into
            # four 128-cluster pieces so its work overlaps the final chunks.
            split_last = cpc >= 512 and nchunk == 8 * nkblk and nkblk >= 1
            if cpc >= 512 and i >= 9 and (i - 9) % 8 == 0:
                kb = (i - 9) // 8
                if kb < (nkblk - 1 if split_last else nkblk):
                    push_g(kb)
            if split_last:
                p = i - 8 * (nkblk - 1) - 4
                if 0 <= p <= 2:
                    push_g(nkblk - 1, 128 * p, 128)
            if split_last and i >= 8 * (nkblk - 1) + 4:
                pop_g()
                pop_g()
            elif i % 2:
                pop_g()

        # ---- epilogue: last chunk's L2+pool, then remaining G work ----
        emit_pool(prev[0], prev[1])
        while g_tasks:
            pop_g()
        if split_last:
            g1in = g_task_b(nkblk - 1, 384, 128,
                            g_task_a(nkblk - 1, 384, 128))
            g_task_c1(nkblk - 1, 384, 128, g1in)
            g_task_c2(nkblk - 1, 384, 128, g1in)
        else:
            first_unpushed = ((nchunk - 10) // 8 + 1) if (cpc >= 512 and nchunk >= 10) else 0
            first_unpushed = min(max(first_unpushed, 0), nkblk)
            for kb in range(first_unpushed, nkblk):
                g1in = g_task_b(kb, 0, kwid, g_task_a(kb, 0, kwid))
                g_task_c1(kb, 0, kwid, g1in)
                g_task_c2(kb, 0, kwid, g1in)

    nc.compile()
    return nc


def _host_pack(points, W0, b0, W1, b1, W2, b2, G0, g0, G1, g1, n4c):
    """Build per-core input maps (host-side layout prep, numpy only)."""
    bf16 = _bf16()
    fp8 = _fp8()
    n = n4c * 4 * NCORES

    # ptsb[4a+f, q] = points[4q+a, f] (f<3); ptsb[4a+3, q] = 1.0
    ptsb = np.ones((16, n // 4), np.float32)
    v = points[:n].reshape(-1, 4, 3).transpose(1, 2, 0)   # [4, 3, n/4]
    for a in range(4):
        ptsb[4 * a:4 * a + 3] = v[a]

    # W0 block-diagonal over 4 points with bias rows: [16, 128]
    w0b = np.zeros((16, 128), np.float32)
    for a in range(4):
        w0b[4 * a:4 * a + 3, 32 * a:32 * a + 32] = W0
        w0b[4 * a + 3, 32 * a:32 * a + 32] = b0
    # W1A/W1B block-diagonal pairs (2-pt/col h1 layout):
    # w1a -> (p0 lo | p1 hi), w1b -> (p2 lo | p3 hi)
    w1a = np.zeros((128, 128), np.float32)
    w1b = np.zeros((128, 128), np.float32)
    for a in range(2):
        w1a[32 * a:32 * a + 32, 64 * a:64 * a + 64] = W1
        w1b[32 * (a + 2):32 * (a + 2) + 32, 64 * a:64 * a + 64] = W1
    # W2 fp8 + fp8 residual stationaries
    w2q = W2.astype(fp8).astype(np.float32)
    w2r = (W2 - w2q).astype(fp8).astype(np.float32)
    w2zb = np.concatenate([w2q, w2r], axis=1).astype(fp8)          # [64,256]
    ddq = np.concatenate([-w2q, w2q], axis=0)                      # [128,128]
    ddr = np.concatenate([-w2r, w2r], axis=0)
    w2dd = np.concatenate([ddq, ddr], axis=1).astype(fp8)          # [128,256]
    d3q = np.concatenate([-w2q, w2q], axis=1)                      # [64,256]
    d3r = np.concatenate([-w2r, w2r], axis=1)
    w2d3 = np.concatenate([d3q, d3r], axis=1).astype(fp8)          # [64,512]
    eye = np.eye(128, dtype=np.float32)
    eye2 = np.concatenate([eye, -eye], axis=1).astype(bf16)        # [128,256]

    common = {
        "w0b": w0b.astype(bf16),
        "w1a": w1a.astype(bf16),
        "w1b": w1b.astype(bf16),
        "w2zb": w2zb,
        "w2dd": w2dd,
        "w2d3": w2d3,
        "eye2": eye2,
        "g0w": G0.astype(bf16),
        "g1lo": G1[:, :128].astype(bf16),
        "g1hi": G1[:, 128:].astype(bf16),
        "b1d": np.tile(b1, 2).reshape(128, 1).astype(np.float32),
        "b2v": b2.reshape(128, 1).astype(np.float32),
        "g0v": g0.reshape(128, 1).astype(np.float32),
        "g1l": g1[:128].reshape(128, 1).astype(np.float32),
        "g1h": g1[128:].reshape(128, 1).astype(np.float32),
    }
    ptsb = ptsb.astype(bf16)
    in_maps = []
    for c in range(NCORES):
        m = dict(common)
        m["ptsb"] = np.ascontiguousarray(ptsb[:, c * n4c:(c + 1) * n4c])
        in_maps.append(m)
    return in_maps


def _numpy_fallback(points, cluster, num_clusters,
                    W0, b0, W1, b1, W2, b2, G0, g0, G1, g1):
    h = points.astype(np.float32)
    for W, b in ((W0, b0), (W1, b1), (W2, b2)):
        h = np.maximum(h @ W + b, 0.0)
    order = np.argsort(cluster, kind="stable")
    cs = cluster[order]
    hs = h[order]
    starts = np.searchsorted(cs, np.arange(num_clusters), side="left")
    counts = np.bincount(cs, minlength=num_clusters)
    safe_starts = np.minimum(starts, max(len(hs) - 1, 0))
    seg = np.maximum.reduceat(hs, safe_starts, axis=0)
    seg[counts == 0] = -np.inf   # match segment_max identity on empties
    pooled = seg
    gx = pooled
    for W, b in ((G0, g0), (G1, g1)):
        gx = np.maximum(gx @ W + b, 0.0)
    return gx.astype(np.float32)


def kernel(**inputs) -> np.ndarray:
    points = np.asarray(inputs["points"], np.float32)
    cluster = np.asarray(inputs["cluster"]).astype(np.int64)
    num_clusters = int(np.asarray(inputs["num_clusters"]))
    W0 = np.asarray(inputs["W0"], np.float32); b0 = np.asarray(inputs["b0"], np.float32)
    W1 = np.asarray(inputs["W1"], np.float32); b1 = np.asarray(inputs["b1"], np.float32)
    W2 = np.asarray(inputs["W2"], np.float32); b2 = np.asarray(inputs["b2"], np.float32)
    G0 = np.asarray(inputs["G0"], np.float32); g0 = np.asarray(inputs["g0"], np.float32)
    G1 = np.asarray(inputs["G1"], np.float32); g1 = np.asarray(inputs["g1"], np.float32)

    expected = (points.shape == (N, 3) and num_clusters == C
                and cluster.shape == (N,))
    if expected:
        # contiguous equal clusters of 32 points, as produced by setup_inputs
        expected = bool(
            np.array_equal(cluster[::PTS], np.arange(C, dtype=np.int64))
            and np.array_equal(cluster, np.repeat(cluster[::PTS], PTS))
        )
    if not expected:
        return _numpy_fallback(points, cluster, num_clusters,
                               W0, b0, W1, b1, W2, b2, G0, g0, G1, g1)

    from concourse.bass_utils import run_bass_kernel_spmd

    if "nc" not in _CACHE:
        _CACHE["nc"] = _build_module(N4C)
    nc = _CACHE["nc"]

    in_maps = _host_pack(points, W0, b0, W1, b1, W2, b2, G0, g0, G1, g1, N4C)
    res = run_bass_kernel_spmd(nc, in_maps, core_ids=list(range(NCORES)))
    outs = []
    for c in range(NCORES):
        o = np.asarray(res.results[c]["outt"]).astype(np.float32)  # [256, CPC]
        outs.append(o.T)                                           # [CPC, 256]
    return np.ascontiguousarray(np.concatenate(outs, axis=0))
